# revision 7
# baseline (speedup 1.0000x reference)
"""Trainium2 Bass kernel for nn_CrossAttentionBlock (cross-attention + gated residual).

Sharding: data-parallel over queries. B=4 batches x 2 halves of L=2048 -> 8 cores,
each core handles 1024 queries of one batch (full attention over that batch's
1024 keys). No collectives needed; weights replicated.

Per-core dataflow (all matmuls bf16 with fp32 PSUM accumulation):
  x [1024q,1024d] --PE transpose--> XT [d,q]
  nar [1024k,512d] --PE transpose--> narT [d,k]
  QT[d_out,q] = Wq^T @ XT   (lhsT=Wq natural)     + bq
  KT[d_out,k] = Wk^T @ narT (lhsT=Wk natural)     + bk
  V [k,d_out] = (narT as lhsT)^T @ Wv + bv, masked rows, + mask column
  per head: scoresT[k,q] = (KT_h as lhsT)^T @ QT_h      (K=64, row-packed pairs)
            A = exp(scoresT/8)          (ACT, PSUM->SBUF, bf16)
            ctxT[d,q] = (V_h as lhsT)^T @ A               (col-packed pairs)
            den[q] = (mask_col as lhsT)^T @ A   (M=1)
            ctxT /= den   (recip + PE broadcast + DVE mul)
  out[q,d] = (ctxT as lhsT)^T @ (g*Wo) + g*bo   (naturally un-transposed)
  y = x + out
"""

import numpy as np
from contextlib import ExitStack

import concourse.bass as bass
import concourse.tile as tile
from concourse import bacc, mybir
from concourse.bass_utils import run_bass_kernel_spmd
from concourse.masks import make_identity

F32 = mybir.dt.float32
BF16 = mybir.dt.bfloat16
I32 = mybir.dt.int32
AF = mybir.ActivationFunctionType

# Problem constants (hardcoded per contest contract).
D_MODEL, D_NAR, N_HEADS, D_HEAD = 1024, 512, 16, 64
B, L, N = 4, 2048, 1024
N_CORES = 8
Q = (B * L) // N_CORES        # 1024 queries per core
NQT = Q // 128                # 8 query tiles
NKT = N // 128                # 8 key tiles
NMT = D_MODEL // 128          # 8 d_model tiles
NNT = D_NAR // 128            # 4 d_nar tiles
SCALE = 1.0 / float(np.sqrt(D_HEAD))


def _build_body(ctx, tc, io):
    nc = tc.nc
    x_d, nar_d, mask_d, wq_d, wk_d, wv_d, wo_d, bq_d, bk_d, bv_d, bo_d, gate_d, y_d = io

    # ---------------- pools ----------------
    consts = ctx.enter_context(tc.tile_pool(name="consts", bufs=1))
    stage_f32 = ctx.enter_context(tc.tile_pool(name="stage_f32", bufs=2))   # [128,1024] f32 staging
    stage_bf = ctx.enter_context(tc.tile_pool(name="stage_bf", bufs=2))     # [128,1024] bf16 staging
    med = ctx.enter_context(tc.tile_pool(name="med", bufs=3))               # [128,4,1024] bf16 (nar-sized)
    big = ctx.enter_context(tc.tile_pool(name="big", bufs=5))               # [128,8,1024] bf16
    vpool = ctx.enter_context(tc.tile_pool(name="vpool", bufs=1))           # V with mask cols
    apool = ctx.enter_context(tc.tile_pool(name="apool", bufs=8))          # exp'd scores, bf16
    denp = ctx.enter_context(tc.tile_pool(name="denp", bufs=2))             # denom rows f32
    recipp = ctx.enter_context(tc.tile_pool(name="recipp", bufs=2))
    bcastp = ctx.enter_context(tc.tile_pool(name="bcastp", bufs=2))
    ypool = ctx.enter_context(tc.tile_pool(name="ypool", bufs=2))

    ps = ctx.enter_context(tc.tile_pool(name="ps", bufs=2, space="PSUM"))       # transient [128,1024] f32
    ps_acc = ctx.enter_context(tc.tile_pool(name="ps_acc", bufs=2, space="PSUM"))  # ctx + den accumulation

    # ---------------- constants ----------------
    # Matmul-feeding rows must sit at base partition 0/32/64 (tile_position rule).
    identones = consts.tile([128, 256], BF16, tag="identones")
    ident = identones[:, 0:128]
    ones_bf = identones[:, 128:256]
    make_identity(nc, ident)
    nc.gpsimd.memset(ones_bf, 1.0)

    # rows_f1/rows_b1: p0 = mask|bv, p32 = bq, p64 = bk
    rows_f1 = consts.tile([128, 2048], F32, tag="rows_f1")
    rows_b1 = consts.tile([128, 2048], BF16, tag="rows_b1")
    # rows_f2/rows_b2: p0 = bo | gate scratch;  rows_b2 p0 = bo | g_row | g_bf
    rows_f2 = consts.tile([128, 2048], F32, tag="rows_f2")
    rows_b2 = consts.tile([128, 2048], BF16, tag="rows_b2")

    mask_i = consts.tile([1, N], I32, tag="mask_i")
    nc.sync.dma_start(mask_i[:], mask_d[:])
    nc.vector.tensor_copy(rows_f1[0:1, 0:1024], mask_i[:])
    nc.sync.dma_start(rows_f1[0:1, 1024:2048], bv_d[:])
    nc.sync.dma_start(rows_f1[32:33, 0:1024], bq_d[:])
    nc.sync.dma_start(rows_f1[64:65, 0:1024], bk_d[:])
    for sl in ((slice(0, 1), slice(0, 2048)), (slice(32, 33), slice(0, 1024)),
               (slice(64, 65), slice(0, 1024))):
        nc.vector.tensor_copy(rows_b1[sl[0], sl[1]], rows_f1[sl[0], sl[1]])
    mask_bf = rows_b1[0:1, 0:1024]
    bv_row = rows_b1[0:1, 1024:2048]
    bq_row = rows_b1[32:33, 0:1024]
    bk_row = rows_b1[64:65, 0:1024]

    nc.sync.dma_start(rows_f2[0:1, 0:1024], bo_d[:])
    nc.vector.tensor_copy(rows_b2[0:1, 0:1024], rows_f2[0:1, 0:1024])
    bo_row = rows_b2[0:1, 0:1024]

    # gate -> g = sigmoid(gate), all on partition 0
    nc.sync.dma_start(rows_f2[0:1, 1024:1025], gate_d[:])
    nc.scalar.activation(rows_f2[0:1, 1025:1026], rows_f2[0:1, 1024:1025], AF.Exp, scale=-1.0)
    nc.scalar.add(rows_f2[0:1, 1026:1027], rows_f2[0:1, 1025:1026], 1.0)
    g_f = rows_f2[0:1, 1027:1028]
    nc.vector.reciprocal(g_f, rows_f2[0:1, 1026:1027])
    g_bf = rows_b2[0:1, 1160:1161]
    with nc.allow_low_precision(reason="gate scalar to bf16 for matmul"):
        nc.vector.tensor_copy(g_bf, g_f)
    g_row = rows_b2[0:1, 1024:1152]
    with nc.allow_low_precision(reason="gate row bf16"):
        nc.vector.tensor_scalar_mul(g_row, ones_bf[0:1, 0:128], g_f)

    # per-partition columns: maskT/bqT/bkT [128,8], g128 [128,1]
    cols_f = consts.tile([128, 32], F32, tag="cols_f")
    maskT = cols_f[:, 0:8]
    bqT = cols_f[:, 8:16]
    bkT = cols_f[:, 16:24]
    g128 = cols_f[:, 24:25]

    ps_small = ps.tile([128, 1024], F32, tag="psbig")
    nc.tensor.matmul(ps_small[:, 0:1], ones_bf[0:1, 0:128], g_bf)
    nc.vector.tensor_copy(g128, ps_small[:, 0:1])

    def row_to_cols(row_bf, col_out):
        p0 = row_bf.base_partition()
        pst = ps.tile([128, 1024], F32, tag="psbig")
        for t in range(NMT):
            nc.tensor.matmul(
                pst[:, t : t + 1],
                row_bf[0:1, t * 128 : (t + 1) * 128],
                ones_bf[p0 : p0 + 1, 0:1],
            )
        nc.vector.tensor_copy(col_out, pst[:, 0:NMT])

    row_to_cols(mask_bf, maskT)
    row_to_cols(bq_row, bqT)
    row_to_cols(bk_row, bkT)

    # ---------------- nar load + transpose ----------------
    narT = med.tile([128, NNT, N], BF16, tag="med")   # narT[p, dt, k] = nar[k, dt*128+p]
    for st in range(NKT):
        stg = stage_f32.tile([128, D_NAR], F32, tag="stg")
        nc.sync.dma_start(stg[:], nar_d[st * 128 : (st + 1) * 128, :])
        stb = stage_bf.tile([128, D_NAR], BF16, tag="stb")
        nc.vector.tensor_copy(stb[:], stg[:])
        ptr = ps.tile([128, 1024], BF16, tag="psbig")
        for dt in range(NNT):
            nc.tensor.transpose(
                ptr[:, dt * 128 : (dt + 1) * 128],
                stb[:, dt * 128 : (dt + 1) * 128],
                ident,
            )
        nc.vector.tensor_copy(
            narT[:, :, st * 128 : (st + 1) * 128],
            ptr[:, 0 : NNT * 128].rearrange("p (t c) -> p t c", c=128),
        )

    # ---------------- V projection (masked, with mask column) ----------------
    wv_bf = med.tile([128, NNT, D_MODEL], BF16, tag="med")
    for kt in range(NNT):
        stg = stage_f32.tile([128, D_MODEL], F32, tag="stg")
        nc.sync.dma_start(stg[:], wv_d[kt * 128 : (kt + 1) * 128, :])
        nc.vector.tensor_copy(wv_bf[:, kt, :], stg[:])

    # V layout: [128, kt, head, 65]; cols 0..63 = V dims (masked), col 64 = mask value
    v_sb = vpool.tile([128, NKT, N_HEADS, D_HEAD + 1], BF16)
    for st in range(NKT):
        pv = ps.tile([128, 1024], F32, tag="psbig")
        for dc in range(2):
            for kt in range(NNT):
                nc.tensor.matmul(
                    pv[:, dc * 512 : (dc + 1) * 512],
                    narT[:, kt, st * 128 : (st + 1) * 128],
                    wv_bf[:, kt, dc * 512 : (dc + 1) * 512],
                    start=(kt == 0),
                    stop=False,
                )
            # + bv (ones row x bias row)
            nc.tensor.matmul(
                pv[:, dc * 512 : (dc + 1) * 512],
                ones_bf[0:1, 0:128],
                bv_row[0:1, dc * 512 : (dc + 1) * 512],
                start=False,
                stop=True,
            )
        for dc in range(2):
            nc.vector.tensor_scalar_mul(
                v_sb[:, st, dc * 8 : (dc + 1) * 8, 0:D_HEAD],
                pv[:, dc * 512 : (dc + 1) * 512].rearrange("p (h d) -> p h d", d=D_HEAD),
                maskT[:, st : st + 1],
            )
        nc.vector.tensor_scalar_mul(
            v_sb[:, st, :, D_HEAD : D_HEAD + 1],
            ones_bf[:, 0:N_HEADS].rearrange("p (h o) -> p h o", o=1),
            maskT[:, st : st + 1],
        )

    # ---------------- K projection ----------------
    wk_bf = med.tile([128, NNT, D_MODEL], BF16, tag="med")
    for kt in range(NNT):
        stg = stage_f32.tile([128, D_MODEL], F32, tag="stg")
        nc.sync.dma_start(stg[:], wk_d[kt * 128 : (kt + 1) * 128, :])
        nc.vector.tensor_copy(wk_bf[:, kt, :], stg[:])

    kT = big.tile([128, NMT, N], BF16, tag="big")   # kT[p, t, k]: d_out = t*128+p
    for t in range(NMT):
        pk = ps.tile([128, 1024], F32, tag="psbig")
        for kc in range(2):
            for kt in range(NNT):
                nc.tensor.matmul(
                    pk[:, kc * 512 : (kc + 1) * 512],
                    wk_bf[:, kt, t * 128 : (t + 1) * 128],
                    narT[:, kt, kc * 512 : (kc + 1) * 512],
                    start=(kt == 0),
                    stop=(kt == NNT - 1),
                )
        nc.vector.tensor_scalar_add(kT[:, t, :], pk[:], bkT[:, t : t + 1])

    # ---------------- x load + transpose ----------------
    xT = big.tile([128, NMT, Q], BF16, tag="big")   # xT[p, dt, q] = x[q, dt*128+p]
    for qt in range(NQT):
        stg = stage_f32.tile([128, D_MODEL], F32, tag="stg")
        nc.sync.dma_start(stg[:], x_d[qt * 128 : (qt + 1) * 128, :])
        stb = stage_bf.tile([128, D_MODEL], BF16, tag="stb")
        nc.vector.tensor_copy(stb[:], stg[:])
        ptr = ps.tile([128, 1024], BF16, tag="psbig")
        for dt in range(NMT):
            nc.tensor.transpose(
                ptr[:, dt * 128 : (dt + 1) * 128],
                stb[:, dt * 128 : (dt + 1) * 128],
                ident,
            )
        nc.vector.tensor_copy(
            xT[:, :, qt * 128 : (qt + 1) * 128],
            ptr[:].rearrange("p (t c) -> p t c", c=128),
        )

    # ---------------- Q projection ----------------
    wq_bf = big.tile([128, NMT, D_MODEL], BF16, tag="big")
    for kt in range(NMT):
        stg = stage_f32.tile([128, D_MODEL], F32, tag="stg")
        nc.sync.dma_start(stg[:], wq_d[kt * 128 : (kt + 1) * 128, :])
        nc.vector.tensor_copy(wq_bf[:, kt, :], stg[:])

    qT = big.tile([128, NMT, Q], BF16, tag="big")
    for t in range(NMT):
        pq = ps.tile([128, 1024], F32, tag="psbig")
        for qc in range(2):
            for kt in range(NMT):
                nc.tensor.matmul(
                    pq[:, qc * 512 : (qc + 1) * 512],
                    wq_bf[:, kt, t * 128 : (t + 1) * 128],
                    xT[:, kt, qc * 512 : (qc + 1) * 512],
                    start=(kt == 0),
                    stop=(kt == NMT - 1),
                )
        nc.vector.tensor_scalar_add(qT[:, t, :], pq[:], bqT[:, t : t + 1])

    # ---------------- Wo load (scaled by g) ----------------
    wo_bf = big.tile([128, NMT, D_MODEL], BF16, tag="big")
    for kt in range(NMT):
        stg = stage_f32.tile([128, D_MODEL], F32, tag="stg")
        nc.sync.dma_start(stg[:], wo_d[kt * 128 : (kt + 1) * 128, :])
        with nc.allow_low_precision(reason="wo*g to bf16"):
            nc.vector.tensor_scalar_mul(wo_bf[:, kt, :], stg[:], g128)

    # ---------------- attention (head pairs; per-head M=65 ctx incl. denominator row) ----------------
    ctxT = big.tile([128, NMT, Q], BF16, tag="big")
    for t in range(NMT):  # head pair t: heads 2t (rows 0:64), 2t+1 (rows 64:128)
        pc_a = ps_acc.tile([D_HEAD + 1, 1024], F32, tag="psacc", name=f"pc_a{t}")
        pc_b = ps_acc.tile([D_HEAD + 1, 1024], F32, tag="psacc", name=f"pc_b{t}")
        pcs = [pc_a, pc_b]
        for kt in range(NKT):
            for half in range(2):
                h = 2 * t + half
                r0 = 64 * half
                psc = ps.tile([128, 1024], F32, tag="psbig")
                for qc in range(2):
                    nc.tensor.matmul(
                        psc[:, qc * 512 : (qc + 1) * 512],
                        kT[r0 : r0 + 64, t, kt * 128 : (kt + 1) * 128],
                        qT[r0 : r0 + 64, t, qc * 512 : (qc + 1) * 512],
                    )
                at = apool.tile([128, 1024], BF16, tag="a")
                nc.scalar.activation(at[:], psc[:], AF.Exp, scale=SCALE)
                # ctx rows 0:64 + denominator row 64, single accumulation group
                for qc in range(2):
                    nc.tensor.matmul(
                        pcs[half][:, qc * 512 : (qc + 1) * 512],
                        v_sb[:, kt, h, 0 : D_HEAD + 1],
                        at[:, qc * 512 : (qc + 1) * 512],
                        start=(kt == 0),
                        stop=(kt == NKT - 1),
                    )
        # normalization
        den_sb = denp.tile([128, 1024], F32, tag="den_sb")
        nc.gpsimd.memset(den_sb[:], 1.0)
        nc.vector.tensor_copy(den_sb[0:1, :], pcs[0][D_HEAD : D_HEAD + 1, :])
        nc.vector.tensor_copy(den_sb[32:33, :], pcs[1][D_HEAD : D_HEAD + 1, :])
        rec = recipp.tile([128, 1024], BF16, tag="rec")
        with nc.allow_low_precision(reason="softmax recip bf16"):
            nc.vector.reciprocal(rec[:], den_sb[:])
        pb = ps.tile([128, 1024], F32, tag="psbig")
        for half in range(2):
            p0 = 32 * half
            for qc in range(2):
                nc.tensor.matmul(
                    pb[64 * half : 64 * half + 64, qc * 512 : (qc + 1) * 512],
                    ones_bf[p0 : p0 + 1, 0:64],
                    rec[p0 : p0 + 1, qc * 512 : (qc + 1) * 512],
                )
        bc = bcastp.tile([128, 1024], BF16, tag="bc")
        with nc.allow_low_precision(reason="recip bcast bf16"):
            nc.vector.tensor_copy(bc[:], pb[:])
        for half in range(2):
            nc.vector.tensor_mul(
                ctxT[64 * half : 64 * half + 64, t, :],
                pcs[half][0:D_HEAD, :],
                bc[64 * half : 64 * half + 64, :],
            )

    # ---------------- out projection + residual ----------------
    for qt in range(NQT):
        po = ps.tile([128, 1024], F32, tag="psbig")
        for dc in range(2):
            for kt in range(NMT):
                nc.tensor.matmul(
                    po[:, dc * 512 : (dc + 1) * 512],
                    ctxT[:, kt, qt * 128 : (qt + 1) * 128],
                    wo_bf[:, kt, dc * 512 : (dc + 1) * 512],
                    start=(kt == 0),
                    stop=False,
                )
            nc.tensor.matmul(
                po[:, dc * 512 : (dc + 1) * 512],
                g_row[0:1, 0:128],
                bo_row[0:1, dc * 512 : (dc + 1) * 512],
                start=False,
                stop=True,
            )
        xre = stage_f32.tile([128, D_MODEL], F32, tag="stg")
        nc.sync.dma_start(xre[:], x_d[qt * 128 : (qt + 1) * 128, :])
        ytile = ypool.tile([128, D_MODEL], F32, tag="y")
        nc.vector.tensor_add(ytile[:], po[:], xre[:])
        nc.sync.dma_start(y_d[qt * 128 : (qt + 1) * 128, :], ytile[:])


_CACHED_NC = None


def build_nc():
    global _CACHED_NC
    if _CACHED_NC is not None:
        return _CACHED_NC
    nc = bacc.Bacc(
        trn_type="TRN2", target_bir_lowering=False, debug=False, num_devices=N_CORES
    )
    io = (
        nc.dram_tensor("x", [Q, D_MODEL], F32, kind="ExternalInput").ap(),
        nc.dram_tensor("nar", [N, D_NAR], F32, kind="ExternalInput").ap(),
        nc.dram_tensor("mask", [1, N], I32, kind="ExternalInput").ap(),
        nc.dram_tensor("wq", [D_MODEL, D_MODEL], F32, kind="ExternalInput").ap(),
        nc.dram_tensor("wk", [D_NAR, D_MODEL], F32, kind="ExternalInput").ap(),
        nc.dram_tensor("wv", [D_NAR, D_MODEL], F32, kind="ExternalInput").ap(),
        nc.dram_tensor("wo", [D_MODEL, D_MODEL], F32, kind="ExternalInput").ap(),
        nc.dram_tensor("bq", [1, D_MODEL], F32, kind="ExternalInput").ap(),
        nc.dram_tensor("bk", [1, D_MODEL], F32, kind="ExternalInput").ap(),
        nc.dram_tensor("bv", [1, D_MODEL], F32, kind="ExternalInput").ap(),
        nc.dram_tensor("bo", [1, D_MODEL], F32, kind="ExternalInput").ap(),
        nc.dram_tensor("gate", [1, 1], F32, kind="ExternalInput").ap(),
        nc.dram_tensor("y", [Q, D_MODEL], F32, kind="ExternalOutput").ap(),
    )
    with tile.TileContext(nc) as tc, ExitStack() as ctx:
        _build_body(ctx, tc, io)
    nc.compile()
    _CACHED_NC = nc
    return nc


def make_in_maps(inputs):
    text_h = np.asarray(inputs["text_h"], dtype=np.float32)
    nar_h = np.asarray(inputs["nar_h"], dtype=np.float32)
    nar_mask = np.asarray(inputs["nar_mask"], dtype=np.int32)
    shared = {
        "wq": np.asarray(inputs["Wq"], dtype=np.float32),
        "wk": np.asarray(inputs["Wk"], dtype=np.float32),
        "wv": np.asarray(inputs["Wv"], dtype=np.float32),
        "wo": np.asarray(inputs["Wo"], dtype=np.float32),
        "bq": np.asarray(inputs["bq"], dtype=np.float32).reshape(1, D_MODEL),
        "bk": np.asarray(inputs["bk"], dtype=np.float32).reshape(1, D_MODEL),
        "bv": np.asarray(inputs["bv"], dtype=np.float32).reshape(1, D_MODEL),
        "bo": np.asarray(inputs["bo"], dtype=np.float32).reshape(1, D_MODEL),
        "gate": np.asarray(inputs["gate"], dtype=np.float32).reshape(1, 1),
    }
    in_maps = []
    for c in range(N_CORES):
        b, half = c // 2, c % 2
        m = dict(shared)
        m["x"] = np.ascontiguousarray(text_h[b, half * Q : (half + 1) * Q, :])
        m["nar"] = np.ascontiguousarray(nar_h[b])
        m["mask"] = np.ascontiguousarray(nar_mask[b].reshape(1, N))
        in_maps.append(m)
    return in_maps


def kernel(**inputs) -> np.ndarray:
    nc = build_nc()
    in_maps = make_in_maps(inputs)
    res = run_bass_kernel_spmd(nc, in_maps, core_ids=list(range(N_CORES)))
    out = np.empty((B, L, D_MODEL), dtype=np.float32)
    for c in range(N_CORES):
        b, half = c // 2, c % 2
        out[b, half * Q : (half + 1) * Q, :] = res.results[c]["y"]
    return out
